# revision 6
# baseline (speedup 1.0000x reference)
"""Trainium2 Bass kernel for nn_RecurrentRetention.

Reference computation (per batch row b, T=2048, DIN=D=1024, fp32):
    Q = xq @ Wq ; K = xk @ Wk ; V = xv @ Wv
    ksum[t] = sum_e K[t, e]
    u[t, :] = ksum[t] * V[t, :]   (u[0, :] forced to 0)
    S[t] = GAMMA * S[t-1] + u[t]  (S[-1] = 0)
    out = Q * S

Kernel strategy (8 NeuronCores, data-parallel over batch — one row per core):
  * Algebraic rewrite: ksum = K.sum(-1) = xk @ rowsum(Wk), so the full
    K = xk @ Wk GEMM is never needed (saves 1/3 of the GEMM FLOPs).
  * Everything on-device runs in a transposed [feature, time] layout so the
    contraction dim (DIN) of the two remaining GEMMs sits on SBUF partitions
    and the time recurrence runs along the free dimension.
  * Q^T and V^T GEMMs in bf16 on TensorE with fp32 PSUM accumulation.
  * ksum row is computed by a thin PE GEMV (wks as the 1-column stationary),
    then broadcast across the 128 partitions with a rank-1 ones matmul.
  * The decay recurrence is a single native DVE prefix scan
    (tensor_tensor_scan: state = gamma * state + u[t], fp32 state) per
    128-feature tile — no sequential inter-tile chain anywhere.
  * Host side only reshapes/casts/slices: transpose inputs to [DIN, T],
    cast to bf16, fold Wk into its row-sum, transpose the output back.

DRAM parameter names/shapes (per core):
  xqT, xkT, xvT : [1024, 2048] bf16   (transposed input row)
  wq,  wv       : [1024, 1024] bf16   (replicated weights)
  wks           : [1024, 1]    bf16   (row-sums of Wk)
  ones          : [1, 128]     bf16
  gam           : [128, 2048]  f32    (GAMMA everywhere; fp32 is required —
                                       bf16 gamma would compound ~0.2% per
                                       step through the recurrence)
  outT (output) : [1024, 2048] f32
"""

import numpy as np

GAMMA = 0.9865
B, T, DIN, D = 8, 2048, 1024, 1024
KT = DIN // 128   # contraction tiles
ET = D // 128     # output-feature tiles
NT = T // 512     # time chunks per PSUM bank
N_CORES = 8

_COMPILED_NC = None


def _build_nc():
    import concourse.bacc as bacc
    import concourse.mybir as mybir
    from concourse import tile

    f32 = mybir.dt.float32
    bf16 = mybir.dt.bfloat16

    nc = bacc.Bacc("TRN2", target_bir_lowering=False, debug=False,
                   num_devices=N_CORES)

    xqT = nc.dram_tensor("xqT", [DIN, T], bf16, kind="ExternalInput")
    xkT = nc.dram_tensor("xkT", [DIN, T], bf16, kind="ExternalInput")
    xvT = nc.dram_tensor("xvT", [DIN, T], bf16, kind="ExternalInput")
    wq = nc.dram_tensor("wq", [DIN, D], bf16, kind="ExternalInput")
    wv = nc.dram_tensor("wv", [DIN, D], bf16, kind="ExternalInput")
    wks = nc.dram_tensor("wks", [DIN, 1], bf16, kind="ExternalInput")
    ones = nc.dram_tensor("ones", [1, 128], bf16, kind="ExternalInput")
    gam = nc.dram_tensor("gam", [128, T], f32, kind="ExternalInput")
    outT = nc.dram_tensor("outT", [D, T], f32, kind="ExternalOutput")

    with tile.TileContext(nc) as tc:
        with (
            tc.tile_pool(name="resident", bufs=1) as res,
            tc.tile_pool(name="xk_stream", bufs=3) as xkp,
            tc.tile_pool(name="u_pool", bufs=2) as up,
            tc.tile_pool(name="s_pool", bufs=2) as sp,
            tc.tile_pool(name="o_pool", bufs=2) as op,
        ):
            # ---- resident loads -------------------------------------------
            wks_t = [res.tile([128, 1], bf16, tag=f"wks{k}", name=f"wks{k}")
                     for k in range(KT)]
            for k in range(KT):
                nc.sync.dma_start(wks_t[k][:], wks[k * 128:(k + 1) * 128, :])
            ones_t = res.tile([1, 128], bf16, tag="ones", name="ones")
            nc.sync.dma_start(ones_t[:], ones[:])
            gam_t = res.tile([128, T], f32, tag="gam", name="gam")
            nc.sync.dma_start(gam_t[:], gam[:])

            xk_t = [xkp.tile([128, T], bf16, tag="xk", name=f"xk{k}")
                    for k in range(KT)]
            for k in range(KT):
                nc.sync.dma_start(xk_t[k][:], xkT[k * 128:(k + 1) * 128, :])

            wv_t = [res.tile([128, D], bf16, tag=f"wv{k}", name=f"wv{k}")
                    for k in range(KT)]
            xv_t = [res.tile([128, T], bf16, tag=f"xv{k}", name=f"xv{k}")
                    for k in range(KT)]
            wq_t = [res.tile([128, D], bf16, tag=f"wq{k}", name=f"wq{k}")
                    for k in range(KT)]
            xq_t = [res.tile([128, T], bf16, tag=f"xq{k}", name=f"xq{k}")
                    for k in range(KT)]
            for k in range(KT):
                nc.sync.dma_start(wv_t[k][:], wv[k * 128:(k + 1) * 128, :])
                nc.sync.dma_start(xv_t[k][:], xvT[k * 128:(k + 1) * 128, :])
            for k in range(KT):
                nc.sync.dma_start(wq_t[k][:], wq[k * 128:(k + 1) * 128, :])
                nc.sync.dma_start(xq_t[k][:], xqT[k * 128:(k + 1) * 128, :])

            ks_row = res.tile([1, T], bf16, tag="ks_row", name="ks_row")
            rep = res.tile([128, T], f32, tag="rep", name="rep")

            # ---- ksum GEMV + partition broadcast --------------------------
            with (
                tc.tile_pool(name="ps_ks", bufs=NT, space="PSUM") as pks,
                tc.tile_pool(name="ps_rep", bufs=2, space="PSUM") as prep,
            ):
                # k-outer so each streamed xk tile is fully consumed before
                # its slot is recycled; the NT accumulators stay live in
                # separate PSUM banks across the whole k loop.
                ks_ps = [pks.tile([1, 512], f32, tag="ksps", name=f"ksps{n}")
                         for n in range(NT)]
                for k in range(KT):
                    for n in range(NT):
                        tsl = slice(n * 512, (n + 1) * 512)
                        nc.tensor.matmul(ks_ps[n][:], wks_t[k][:],
                                         xk_t[k][:, tsl],
                                         start=(k == 0), stop=(k == KT - 1))
                for n in range(NT):
                    tsl = slice(n * 512, (n + 1) * 512)
                    # fp32 PSUM -> bf16 SBUF row (ScalarE keeps DVE free)
                    nc.scalar.copy(ks_row[:, tsl], ks_ps[n][:])
                for n in range(NT):
                    tsl = slice(n * 512, (n + 1) * 512)
                    rep_ps = prep.tile([128, 512], f32, tag="repps",
                                       name=f"repps{n}")
                    nc.tensor.matmul(rep_ps[:], ones_t[:], ks_row[:, tsl],
                                     start=True, stop=True)
                    nc.vector.tensor_copy(rep[:, tsl], rep_ps[:])

            # ---- main e-tile loop -----------------------------------------
            with (
                tc.tile_pool(name="ps_v", bufs=3, space="PSUM") as pv,
                tc.tile_pool(name="ps_q", bufs=3, space="PSUM") as pq,
            ):
                for e in range(ET):
                    esl = slice(e * 128, (e + 1) * 128)
                    u_e = up.tile([128, T], bf16, tag="u", name=f"u{e}")
                    for n in range(NT):
                        tsl = slice(n * 512, (n + 1) * 512)
                        v_ps = pv.tile([128, 512], f32, tag="vps",
                                       name=f"vps{e}_{n}")
                        for k in range(KT):
                            nc.tensor.matmul(v_ps[:], wv_t[k][:, esl],
                                             xv_t[k][:, tsl],
                                             start=(k == 0),
                                             stop=(k == KT - 1))
                        # u = V^T * ksum  (PSUM x SBUF -> SBUF bf16)
                        nc.vector.tensor_mul(u_e[:, tsl], v_ps[:],
                                             rep[:, tsl])
                    # u[:, 0] = 0: t=0 never contributes to the recurrence
                    nc.gpsimd.memset(u_e[:, 0:1], 0.0)
                    s_e = sp.tile([128, T], f32, tag="s", name=f"s{e}")
                    nc.vector.tensor_tensor_scan(
                        s_e[:], gam_t[:], u_e[:], 0.0,
                        op0=mybir.AluOpType.mult, op1=mybir.AluOpType.add)
                    o_e = op.tile([128, T], f32, tag="o", name=f"o{e}")
                    for n in range(NT):
                        tsl = slice(n * 512, (n + 1) * 512)
                        q_ps = pq.tile([128, 512], f32, tag="qps",
                                       name=f"qps{e}_{n}")
                        for k in range(KT):
                            nc.tensor.matmul(q_ps[:], wq_t[k][:, esl],
                                             xq_t[k][:, tsl],
                                             start=(k == 0),
                                             stop=(k == KT - 1))
                        nc.vector.tensor_mul(o_e[:, tsl], q_ps[:],
                                             s_e[:, tsl])
                    nc.sync.dma_start(outT[esl, :], o_e[:])

    nc.compile()
    return nc


def _get_nc():
    global _COMPILED_NC
    if _COMPILED_NC is None:
        _COMPILED_NC = _build_nc()
    return _COMPILED_NC


def _make_in_maps(xq, xk, xv, Wq, Wk, Wv):
    import ml_dtypes

    bf16 = ml_dtypes.bfloat16
    wq_b = Wq.astype(bf16)
    wv_b = Wv.astype(bf16)
    wks = Wk.sum(axis=1, dtype=np.float32).astype(bf16).reshape(DIN, 1)
    ones = np.ones((1, 128), dtype=bf16)
    gam = np.full((128, T), GAMMA, dtype=np.float32)

    in_maps = []
    for c in range(N_CORES):
        in_maps.append({
            "xqT": np.ascontiguousarray(xq[c].T).astype(bf16),
            "xkT": np.ascontiguousarray(xk[c].T).astype(bf16),
            "xvT": np.ascontiguousarray(xv[c].T).astype(bf16),
            "wq": wq_b,
            "wv": wv_b,
            "wks": wks,
            "ones": ones,
            "gam": gam,
        })
    return in_maps


def run_on_hw(xq, xk, xv, Wq, Wk, Wv, trace=False):
    """Returns (output [B,T,D] fp32, BassKernelResults)."""
    from concourse.bass_utils import run_bass_kernel_spmd

    nc = _get_nc()
    in_maps = _make_in_maps(
        np.asarray(xq), np.asarray(xk), np.asarray(xv),
        np.asarray(Wq), np.asarray(Wk), np.asarray(Wv))
    res = run_bass_kernel_spmd(nc, in_maps, list(range(N_CORES)), trace=trace)
    out = np.empty((B, T, D), dtype=np.float32)
    for c in range(N_CORES):
        out[c] = res.results[c]["outT"].T
    return out, res


def kernel(xq, xk, xv, Wq, Wk, Wv):
    out, _ = run_on_hw(xq, xk, xv, Wq, Wk, Wv, trace=False)
    return out
